# revision 5
# baseline (speedup 1.0000x reference)
"""AttentionDecoder on 8 Trainium2 NeuronCores.

Sharding:
  - attention phase: data-parallel over batch B (each core: 32 rows of B=256)
  - LSTM stack: tensor-parallel over the hidden dim (each core: a 128-wide
    h-slice of all four i/f/g/o gates, for all 4 layers); x / h exchanged
    via AllGather through Shared DRAM between layers
  - final_output: per-core partial over the h-slice contraction, summed on host
"""

import numpy as np

S1, S2, B, H, E, IN, OUT, NL = 8, 32, 256, 1024, 512, 2, 2, 4
NC = 8            # cores
BL = B // NC      # 32  local batch rows (attention shard)
HL = H // NC      # 128 local h-slice (LSTM shard)
EH = E + H        # 1536
IG = S2 // 4      # 8 i-groups of 4 (tile packing: partition = (i_loc, b))

_BUILT = {}


def _build_program():
    import concourse.bass as bass
    import concourse.mybir as mybir
    import concourse.tile as tile

    f32 = mybir.dt.float32
    AF = mybir.ActivationFunctionType
    ALU = mybir.AluOpType

    nc = bass.Bass()

    # ---------------- DRAM parameters (per-core values supplied via in_maps)
    def inp(name, shape):
        return nc.dram_tensor(name, shape, f32, kind="ExternalInput")

    ak = inp("ak", [S1, IG, 128, H])       # A slice, tiles [(i_loc,b), h]
    htop = inp("htop", [BL, H])            # h0[0] local batch rows
    wyr = inp("wyr", [128, H])             # w_y replicated over partitions
    whr = inp("whr", [BL, H])              # w_h replicated
    batn = inp("batn", [128, 1])           # b_attn replicated
    s2m = inp("s2m", [BL, 128])            # S2[b, p] = 1 if p % 32 == b
    wsel = inp("wsel", [128, BL])          # 8.0 if p % 32 == b  (S1 folded)
    iden = inp("iden", [128, 128])
    w0r = inp("w0r", [BL, E])              # W_emb[:,0] replicated
    w1r = inp("w1r", [BL, E])
    ber = inp("ber", [BL, E])              # b_emb replicated
    xin_l = inp("xin_l", [BL, IN])         # input local rows
    wih0 = inp("wih0", [EH, 4 * HL])       # Wih0[rows_k,:].T, cols = (gate, h)
    whhT = inp("whhT", [NL, H, 4 * HL])    # Whh[l][rows_k,:].T
    wihT = inp("wihT", [NL - 1, H, 4 * HL])
    bias = inp("bias", [NL, HL, 4])        # (bih+bhh)[rows_k] as [h_loc, gate]
    h0t = inp("h0t", [NL, H, B])           # h0[l].T full
    c0t = inp("c0t", [NL, HL, B])          # c0[l][:, h-slice].T
    woutT = inp("woutT", [HL, OUT])        # W_out[:, h-slice].T
    borep = inp("borep", [128, OUT])       # b_out / NC replicated

    norm_o = nc.dram_tensor("norm_o", [BL, S1 * S2], f32, kind="ExternalOutput")
    hn_o = nc.dram_tensor("hn_o", [NL, B, HL], f32, kind="ExternalOutput")
    cn_o = nc.dram_tensor("cn_o", [NL, B, HL], f32, kind="ExternalOutput")
    fo_o = nc.dram_tensor("fo_o", [B, OUT], f32, kind="ExternalOutput")

    # Shared DRAM tensors for collectives
    xfull_sh = nc.dram_tensor("xfull_sh", [NC, BL, EH], f32, addr_space="Shared")
    hfull_sh = [
        nc.dram_tensor(f"hfull_sh{l}", [NC, HL, B], f32, addr_space="Shared")
        for l in range(NL - 1)
    ]

    groups = [list(range(NC))]

    with tile.TileContext(nc) as tc:
        with (
            tc.tile_pool(name="consts", bufs=1) as consts,
            tc.tile_pool(name="a_pool", bufs=4) as a_pool,
            tc.tile_pool(name="trash", bufs=1) as trash_pool,
            tc.tile_pool(name="small", bufs=1) as small,
            tc.tile_pool(name="ev", bufs=4) as evp,
            tc.tile_pool(name="wt", bufs=20) as wtp,
            tc.tile_pool(name="hx", bufs=1) as hxp,
            tc.tile_pool(name="gate", bufs=1) as gatep,
            tc.tile_pool(name="ps_gate", bufs=4, space="PSUM") as ps_gate,
            tc.tile_pool(name="ps_misc", bufs=2, space="PSUM") as ps_misc,
            tc.tile_pool(name="dram", bufs=1, space="DRAM") as dram,
        ):
            # ---------------- constants
            wyr_sb = consts.tile([128, H], f32)
            nc.sync.dma_start(wyr_sb[:], wyr[:])
            iden_sb = consts.tile([128, 128], f32)
            nc.sync.dma_start(iden_sb[:], iden[:])
            s2_sb = consts.tile([BL, 128], f32)
            nc.sync.dma_start(s2_sb[:], s2m[:])
            wsel_sb = consts.tile([128, BL], f32)
            nc.sync.dma_start(wsel_sb[:], wsel[:])

            # ---------------- phase A: scores  (TTR over h on DVE)
            scores_T = consts.tile([128, S1 * IG], f32)   # [(i_loc,b), (j,ig)]
            for j in range(S1):
                for g in range(IG):
                    a_t = a_pool.tile([128, H], f32, tag="a", name=f"a{j}_{g}")
                    nc.sync.dma_start(a_t[:], ak[j, g])
                    tr = trash_pool.tile([128, H], f32, tag="trash")
                    nc.vector.scalar_tensor_tensor(
                        out=tr[:],
                        in0=a_t[:],
                        scalar=1.0,
                        in1=wyr_sb[:],
                        op0=ALU.bypass,
                        op1=ALU.mult,
                        accum_out=scores_T[:, j * IG + g : j * IG + g + 1],
                    )

            # h-dependent score term: htop @ w_h + b_attn  -> [BL, 1]
            htop_sb = small.tile([BL, H], f32, tag="htop")
            nc.sync.dma_start(htop_sb[:], htop[:])
            whr_sb = small.tile([BL, H], f32, tag="whr")
            nc.sync.dma_start(whr_sb[:], whr[:])
            batn_sb = small.tile([128, 1], f32, tag="batn")
            nc.sync.dma_start(batn_sb[:], batn[:])
            hterm = small.tile([BL, 1], f32, tag="hterm")
            tr2 = trash_pool.tile([128, H], f32, tag="trash", name="tr2")
            nc.vector.scalar_tensor_tensor(
                out=tr2[:BL],
                in0=htop_sb[:],
                scalar=1.0,
                in1=whr_sb[:],
                op0=ALU.bypass,
                op1=ALU.mult,
                accum_out=hterm[:],
            )
            # broadcast to 128 partitions via selector matmul, add b_attn
            hterm_ps = ps_misc.tile([128, 1], f32, tag="misc")
            nc.tensor.matmul(hterm_ps[:], s2_sb[:], hterm[:])
            hterm_b = small.tile([128, 1], f32, tag="hterm_b")
            nc.vector.tensor_add(hterm_b[:], hterm_ps[:], batn_sb[:])

            # exp(scores + hterm)
            scoresE = consts.tile([128, S1 * IG], f32)
            nc.scalar.activation(scoresE[:], scores_T[:], AF.Exp, bias=hterm_b[:])

            # permute to [b, (j, ig, i_loc)] via 4 identity-slice matmuls
            sEb = consts.tile([BL, S1 * S2], f32)
            sEb_v = sEb[:].rearrange("p (j g q) -> p j g q", g=IG, q=4)
            for q in range(4):
                oq = ps_misc.tile([BL, S1 * IG], f32, tag="misc")
                nc.tensor.matmul(oq[:], iden_sb[:, q * BL : (q + 1) * BL], scoresE[:])
                nc.vector.tensor_copy(
                    sEb_v[:, :, :, q],
                    oq[:].rearrange("p (j g) -> p j g", g=IG),
                )

            sumexp = small.tile([BL, 1], f32, tag="sumexp")
            nc.vector.tensor_reduce(
                sumexp[:], sEb[:], axis=mybir.AxisListType.X, op=ALU.add
            )
            recip = small.tile([BL, 1], f32, tag="recip")
            nc.vector.reciprocal(recip[:], sumexp[:])

            normed = consts.tile([BL, S1 * S2], f32)
            nc.vector.tensor_scalar_mul(normed[:], sEb[:], recip[:])
            nc.sync.dma_start(norm_o[:], normed[:])

            # recip broadcast to 128 partitions
            recip_ps = ps_misc.tile([128, 1], f32, tag="misc")
            nc.tensor.matmul(recip_ps[:], s2_sb[:], recip[:])
            recip_b = small.tile([128, 1], f32, tag="recip_b")
            nc.vector.tensor_copy(recip_b[:], recip_ps[:])

            # attention weights for the last j block: [(i_loc,b), ig]
            wcol = small.tile([128, IG], f32, tag="wcol")
            nc.vector.tensor_scalar_mul(
                wcol[:], scoresE[:, (S1 - 1) * IG : S1 * IG], recip_b[:]
            )

            # weighted sum of A7 rows -> acc [(i_loc,b), h]
            acc = consts.tile([128, H], f32)
            term = trash_pool.tile([128, H], f32, tag="trash", name="term")
            for g in range(IG):
                a7_t = a_pool.tile([128, H], f32, tag="a", name=f"a7r{g}")
                nc.sync.dma_start(a7_t[:], ak[S1 - 1, g])
                if g == 0:
                    nc.vector.tensor_scalar_mul(acc[:], a7_t[:], wcol[:, 0:1])
                else:
                    nc.vector.tensor_scalar_mul(term[:], a7_t[:], wcol[:, g : g + 1])
                    nc.vector.tensor_add(acc[:], acc[:], term[:])

            # reduce i_loc groups (x8 for the aliasing bug) -> final_attn [b, h]
            x_local = consts.tile([BL, EH], f32)
            for half in range(2):
                app_ps = ps_misc.tile([BL, H // 2], f32, tag="misc")
                nc.tensor.matmul(
                    app_ps[:], wsel_sb[:], acc[:, half * (H // 2) : (half + 1) * (H // 2)]
                )
                nc.vector.tensor_copy(
                    x_local[:, half * (H // 2) : (half + 1) * (H // 2)], app_ps[:]
                )

            # embedded = relu(input @ W_emb.T + b_emb) -> x_local[:, H:]
            in_sb = small.tile([BL, IN], f32, tag="in_sb")
            nc.sync.dma_start(in_sb[:], xin_l[:])
            w0_sb = small.tile([BL, E], f32, tag="w0")
            nc.sync.dma_start(w0_sb[:], w0r[:])
            w1_sb = small.tile([BL, E], f32, tag="w1")
            nc.sync.dma_start(w1_sb[:], w1r[:])
            be_sb = small.tile([BL, E], f32, tag="be")
            nc.sync.dma_start(be_sb[:], ber[:])
            t0 = small.tile([BL, E], f32, tag="t0")
            nc.vector.tensor_scalar_mul(t0[:], w0_sb[:], in_sb[:, 0:1])
            t1 = small.tile([BL, E], f32, tag="t1")
            nc.vector.tensor_scalar_mul(t1[:], w1_sb[:], in_sb[:, 1:2])
            nc.vector.tensor_add(t0[:], t0[:], t1[:])
            nc.vector.tensor_add(t0[:], t0[:], be_sb[:])
            nc.scalar.activation(x_local[:, H:], t0[:], AF.Relu)

            # ---------------- AllGather x
            xb = dram.tile([BL, EH], f32, tag="xb")
            nc.sync.dma_start(xb[:], x_local[:])
            nc.gpsimd.collective_compute(
                "AllGather",
                ALU.bypass,
                replica_groups=groups,
                ins=[xb.opt()],
                outs=[xfull_sh[:]],
            )

            # read back + transpose -> xT chunks [e(128), B]
            xfull = xfull_sh[:].rearrange("k b e -> (k b) e")
            xT = []  # 12 tiles [128, B]
            for ec in range(EH // 128):
                xT.append(hxp.tile([128, B], f32, tag=f"xh{ec}", name=f"xT{ec}"))
            for bc in range(2):
                xf_sb = small.tile([128, EH], f32, tag="xf")
                nc.sync.dma_start(xf_sb[:], xfull[bc * 128 : (bc + 1) * 128, :])
                for ec in range(EH // 128):
                    tp = ps_misc.tile([128, 128], f32, tag="misc")
                    nc.tensor.transpose(
                        tp[:], xf_sb[:, ec * 128 : (ec + 1) * 128], iden_sb[:]
                    )
                    nc.vector.tensor_copy(
                        xT[ec][:, bc * 128 : (bc + 1) * 128], tp[:]
                    )

            # ---------------- LSTM layers (TP over gate/hidden dim)
            hT_prev = None  # list of 8 chunks [128, B] for layers >= 1
            for l in range(NL):
                # load weights (transposed, sliced) for this layer
                if l == 0:
                    wih_chunks = EH // 128
                    wih_src = wih0
                else:
                    wih_chunks = H // 128
                    wih_src = wihT[l - 1]
                wih_sb = []
                for c in range(wih_chunks):
                    w = wtp.tile([128, 4 * HL], f32, tag="w", name=f"w{l}")
                    nc.sync.dma_start(w[:], wih_src[c * 128 : (c + 1) * 128, :])
                    wih_sb.append(w)
                whh_sb = []
                for c in range(H // 128):
                    w = wtp.tile([128, 4 * HL], f32, tag="w", name=f"w{l}")
                    nc.sync.dma_start(w[:], whhT[l, c * 128 : (c + 1) * 128, :])
                    whh_sb.append(w)
                h0t_sb = []
                for c in range(H // 128):
                    h = hxp.tile([128, B], f32, tag=f"h0t{c}", name=f"h0t{l}_{c}")
                    nc.sync.dma_start(h[:], h0t[l, c * 128 : (c + 1) * 128, :])
                    h0t_sb.append(h)
                c0t_sb = gatep.tile([HL, B], f32, tag="c0t")
                nc.sync.dma_start(c0t_sb[:], c0t[l])
                bias_sb = small.tile([HL, 4], f32, tag="bias")
                nc.sync.dma_start(bias_sb[:], bias[l])

                xin_chunks = xT if l == 0 else hT_prev
                g_ps = [ps_gate.tile([HL, B], f32, tag="g", name=f"g{l}_{i}") for i in range(4)]
                for g in range(4):
                    gsl = slice(g * HL, (g + 1) * HL)
                    n_h = H // 128
                    for c in range(n_h):
                        nc.tensor.matmul(
                            g_ps[g][:],
                            whh_sb[c][:, gsl],
                            h0t_sb[c][:],
                            start=(c == 0),
                            stop=False,
                        )
                    for c in range(wih_chunks):
                        nc.tensor.matmul(
                            g_ps[g][:],
                            wih_sb[c][:, gsl],
                            xin_chunks[c][:],
                            start=False,
                            stop=(c == wih_chunks - 1),
                        )

                sig_i = gatep.tile([HL, B], f32, tag="sig_i")
                nc.scalar.activation(sig_i[:], g_ps[0][:], AF.Sigmoid, bias=bias_sb[:, 0:1])
                sig_f = gatep.tile([HL, B], f32, tag="sig_f")
                nc.scalar.activation(sig_f[:], g_ps[1][:], AF.Sigmoid, bias=bias_sb[:, 1:2])
                tanh_g = gatep.tile([HL, B], f32, tag="tanh_g")
                nc.scalar.activation(tanh_g[:], g_ps[2][:], AF.Tanh, bias=bias_sb[:, 2:3])
                sig_o = gatep.tile([HL, B], f32, tag="sig_o")
                nc.scalar.activation(sig_o[:], g_ps[3][:], AF.Sigmoid, bias=bias_sb[:, 3:4])

                cT = gatep.tile([HL, B], f32, tag="cT")
                nc.vector.tensor_mul(cT[:], sig_f[:], c0t_sb[:])
                nc.vector.tensor_mul(tanh_g[:], sig_i[:], tanh_g[:])
                nc.vector.tensor_add(cT[:], cT[:], tanh_g[:])
                tanh_c = gatep.tile([HL, B], f32, tag="tanh_c")
                nc.scalar.activation(tanh_c[:], cT[:], AF.Tanh)
                hT = gatep.tile([HL, B], f32, tag="hT")
                nc.vector.tensor_mul(hT[:], sig_o[:], tanh_c[:])

                # write h_new / c_new slices (transpose back to [b, h_loc])
                for bc in range(2):
                    for src, dst in ((cT, cn_o), (hT, hn_o)):
                        tp = ps_misc.tile([128, HL], f32, tag="misc")
                        nc.tensor.transpose(
                            tp[:], src[:, bc * 128 : (bc + 1) * 128], iden_sb[:]
                        )
                        ev = evp.tile([128, HL], f32, tag="ev", name=f"ev{l}_{bc}")
                        nc.vector.tensor_copy(ev[:], tp[:])
                        nc.sync.dma_start(dst[l, bc * 128 : (bc + 1) * 128, :], ev[:])

                if l < NL - 1:
                    hb = dram.tile([HL, B], f32, tag=f"hb{l}")
                    nc.sync.dma_start(hb[:], hT[:])
                    nc.gpsimd.collective_compute(
                        "AllGather",
                        ALU.bypass,
                        replica_groups=groups,
                        ins=[hb.opt()],
                        outs=[hfull_sh[l][:]],
                    )
                    hT_prev = []
                    for c in range(H // 128):
                        t = hxp.tile([128, B], f32, tag=f"xh{c}", name=f"hTc{l}_{c}")
                        nc.sync.dma_start(t[:], hfull_sh[l][c])
                        hT_prev.append(t)
                else:
                    # final_output partial: h3(local slice) @ W_out.T + b_out/NC
                    wout_sb = small.tile([HL, OUT], f32, tag="wout")
                    nc.sync.dma_start(wout_sb[:], woutT[:])
                    bo_sb = small.tile([128, OUT], f32, tag="bo")
                    nc.sync.dma_start(bo_sb[:], borep[:])
                    for bc in range(2):
                        fo_ps = ps_misc.tile([128, OUT], f32, tag="misc")
                        nc.tensor.matmul(
                            fo_ps[:], hT[:, bc * 128 : (bc + 1) * 128], wout_sb[:]
                        )
                        fo_sb = small.tile([128, OUT], f32, tag="fo")
                        nc.vector.tensor_add(fo_sb[:], fo_ps[:], bo_sb[:])
                        nc.sync.dma_start(
                            fo_o[bc * 128 : (bc + 1) * 128, :], fo_sb[:]
                        )

    return nc


def _split_multiwait(nc, max_waits=1):
    """This walrus build rejects >1 embedded sem-wait on CTRL-class
    instructions (the Tile tail drain carries one per live engine/queue).
    Hoist extras onto dedicated single-wait Drains just before."""
    import concourse.mybir as mybir

    for f in nc.m.functions:
        for b in f.blocks:
            new_list, changed = [], False
            for ins in b.instructions:
                si = ins.sync_info
                if si is not None and len(si.on_wait) > max_waits:
                    waits = list(si.on_wait)
                    for j, wt in enumerate(waits[:-max_waits]):
                        d = mybir.InstDrain(
                            name=f"{ins.name}_wsplit{j}", ins=[], outs=[]
                        )
                        d.engine = ins.engine
                        d.sync_info = mybir.SyncInfo(on_wait=[wt], on_update=[])
                        new_list.append(d)
                        nc.register_instruction(d, overwrite=True)
                    del si.on_wait[: len(waits) - max_waits]
                    changed = True
                new_list.append(ins)
            if changed:
                b.instructions = new_list


def _get_program():
    if "nc" not in _BUILT:
        nc = _build_program()
        _split_multiwait(nc)
        _BUILT["nc"] = nc
    return _BUILT["nc"]


def _prep_in_maps(input, h0, c0, all_outputs, W_emb, b_emb, W_attn, b_attn,
                  Wih0, Whh0, bih0, bhh0, Wih_r, Whh_r, bih_r, bhh_r,
                  W_out, b_out):
    f = np.float32
    asf = lambda a: np.ascontiguousarray(a, dtype=f)

    w_y = W_attn[0, H:]
    w_h = W_attn[0, :H]
    wyr = asf(np.tile(w_y, (128, 1)))
    whr = asf(np.tile(w_h, (BL, 1)))
    batn = np.full((128, 1), b_attn[0], dtype=f)
    iden = np.eye(128, dtype=f)
    p = np.arange(128)
    s2m = asf((p[None, :] % BL) == np.arange(BL)[:, None])
    wsel = asf(((p[:, None] % BL) == np.arange(BL)[None, :]) * float(S1))
    w0r = asf(np.tile(W_emb[:, 0], (BL, 1)))
    w1r = asf(np.tile(W_emb[:, 1], (BL, 1)))
    ber = asf(np.tile(b_emb, (BL, 1)))
    borep = asf(np.tile(b_out / NC, (128, 1)))

    bsum0 = bih0 + bhh0                       # [4H]
    bsum_r = bih_r + bhh_r                    # [NL-1, 4H]
    h0t = asf(h0.transpose(0, 2, 1))          # [NL, H, B]

    in_maps = []
    for k in range(NC):
        b0, hs = BL * k, HL * k
        ak = asf(
            all_outputs[:, :, b0 : b0 + BL, :]
            .reshape(S1, IG, 4, BL, H)
            .reshape(S1, IG, 128, H)
        )
        rows = np.concatenate([g * H + hs + np.arange(HL) for g in range(4)])
        wih0_k = asf(Wih0[rows, :].T)                       # [EH, 4*HL]
        whhT_k = np.empty((NL, H, 4 * HL), dtype=f)
        whhT_k[0] = Whh0[rows, :].T
        for l in range(1, NL):
            whhT_k[l] = Whh_r[l - 1][rows, :].T
        wihT_k = np.empty((NL - 1, H, 4 * HL), dtype=f)
        for l in range(NL - 1):
            wihT_k[l] = Wih_r[l][rows, :].T
        bias_k = np.empty((NL, HL, 4), dtype=f)
        bias_k[0] = bsum0[rows].reshape(4, HL).T
        for l in range(1, NL):
            bias_k[l] = bsum_r[l - 1][rows].reshape(4, HL).T
        c0t_k = asf(c0[:, :, hs : hs + HL].transpose(0, 2, 1))   # [NL, HL, B]
        in_maps.append({
            "ak": ak,
            "htop": asf(h0[0, b0 : b0 + BL, :]),
            "wyr": wyr, "whr": whr, "batn": batn,
            "s2m": s2m, "wsel": wsel, "iden": iden,
            "w0r": w0r, "w1r": w1r, "ber": ber,
            "xin_l": asf(input[b0 : b0 + BL, :]),
            "wih0": wih0_k, "whhT": asf(whhT_k), "wihT": asf(wihT_k),
            "bias": asf(bias_k), "h0t": h0t, "c0t": c0t_k,
            "woutT": asf(W_out[:, hs : hs + HL].T),
            "borep": borep,
        })
    return in_maps


def _run(in_maps, trace=False):
    from concourse.bass_utils import run_bass_kernel_spmd

    nc = _get_program()
    return run_bass_kernel_spmd(nc, in_maps, list(range(NC)), trace=trace)


def _assemble(results):
    final_output = np.sum([r["fo_o"] for r in results], axis=0, dtype=np.float64)
    final_output = final_output.astype(np.float32)
    h_new = np.concatenate([r["hn_o"] for r in results], axis=2)
    c_new = np.concatenate([r["cn_o"] for r in results], axis=2)
    normalized = np.concatenate([r["norm_o"] for r in results], axis=0)
    return final_output, h_new, c_new, normalized


def kernel(**inputs):
    in_maps = _prep_in_maps(**inputs)
    res = _run(in_maps, trace=False)
    return _assemble(res.results)


def kernel_traced(**inputs):
    """Like kernel() but with NTFF profiling; returns (outputs, exec_time_ns)."""
    in_maps = _prep_in_maps(**inputs)
    res = _run(in_maps, trace=True)
    return _assemble(res.results), res.exec_time_ns
